# revision 24
# baseline (speedup 1.0000x reference)
"""DeepSets ensemble (segment mean-pool + BN MLP) on 8 TRN2 NeuronCores.

Strategy (data-parallel, per sharding hint):
 - events are split 1024/core; each core's points are bin-packed (FFD) into
   512-pt groups of whole events (<=6 per group), zero-padded, so the ragged
   segment-sum becomes a block matmul against a host-built selector S whose
   entries are 1/len (mean pooling folded into the matmul).
 - per core: phi1 (x [16,512]-tiles bf16 -> PSUM), fused-relu PSUM->SBUF
   copies (alternating DVE/ACT), phi2 with the h1-tile as the stationary
   matmul operand producing h2^T [128pts, 64] tiles, then pooling matmuls
   h2^T x S (12 merged A/B slot cols per k-tile) accumulating z in PSUM.
 - MLP sharded over slots, computed in bf16; BatchNorm uses two tiny
   AllReduces of (sum, sum-of-squares); a dummy AllReduce at kernel start
   warms the collective stream; v1 + its stats are pipelined into the main
   loop per flushed PSUM bank; empty-slot contributions are corrected
   analytically after the second AllReduce.
 - host scatters x_scalar / gathers y through the slot<->event map.
"""
import sys
import numpy as np
import ml_dtypes
from contextlib import ExitStack

sys.path.insert(0, "/opt/trn_rl_repo")

import concourse.bacc as bacc
import concourse.tile as tile
from concourse import mybir
from concourse import bass_utils

BF16 = mybir.dt.bfloat16
F32 = mybir.dt.float32
AX = mybir.AxisListType
OP = mybir.AluOpType
ACTF = mybir.ActivationFunctionType

N_CORES = 8
C_IN = 16
F = 64
S_SCALAR = 8
M1, M2 = 128, 64
G = 512
SLOTS = 6
NB = 42          # chunks per pooling PSUM bank (12 cols each -> 504)
EPS = 1e-5


def _plan_groups(lengths, b_total):
    e_per_core = b_total // N_CORES
    cores = []
    for c in range(N_CORES):
        evs = sorted(range(c * e_per_core, (c + 1) * e_per_core),
                     key=lambda e: -int(lengths[e]))
        groups, space = [], []
        for e in evs:
            l = int(lengths[e])
            assert 0 < l <= G
            placed = False
            for gi in range(len(groups)):
                if space[gi] >= l and len(groups[gi]) < SLOTS:
                    groups[gi].append(e)
                    space[gi] -= l
                    placed = True
                    break
            if not placed:
                groups.append([e])
                space.append(G - l)
        cores.append(groups)
    ng = max(len(g) for g in cores)
    if ng % 2:
        ng += 1
    for g in cores:
        while len(g) < ng:
            g.append([])
    return cores, ng


def _prep_core(x, x_scalar, lengths, offsets, groups, ng):
    nchunk = ng // 2
    p_pad = ng * G
    sl = ng * SLOTS
    xb = np.zeros((2 * C_IN, p_pad // 2), dtype=np.float32)
    # merged A/B selector: per chunk j, 4 k-tile blocks of 12 cols
    # (6 A-slot cols then 6 B-slot cols), entries are 1/len
    s_mat = np.zeros((128, nchunk * 4 * 12), dtype=np.float32)
    xsT = np.zeros((S_SCALAR, sl), dtype=np.float32)
    slot_events = np.full(sl, -1, dtype=np.int64)
    for j in range(nchunk):
        for half, g_idx in ((0, j), (1, nchunk + j)):
            evs = groups[g_idx]
            col0 = 512 * j
            row0 = C_IN * half
            pt = 0
            for i, e in enumerate(evs):
                l = int(lengths[e])
                o = int(offsets[e])
                xb[row0:row0 + C_IN, col0 + pt: col0 + pt + l] = x[:, o:o + l]
                p_arr = np.arange(pt, pt + l)
                s_mat[p_arr % 128,
                      (4 * j + p_arr // 128) * 12 + 6 * half + i] = 1.0 / l
                slot = SLOTS * g_idx + i
                xsT[:, slot] = x_scalar[e]
                slot_events[slot] = e
                pt += l
    return {
        "xb": np.ascontiguousarray(xb.astype(ml_dtypes.bfloat16)),
        "S": np.ascontiguousarray(s_mat.astype(ml_dtypes.bfloat16)),
        "xsT": np.ascontiguousarray(xsT.astype(ml_dtypes.bfloat16)),
        "slot_events": slot_events,
    }


def _build_nc(ng, sl, b_total):
    nchunk = ng // 2
    p_pad = ng * G
    n_empty = float(N_CORES * sl - b_total)
    inv_b = 1.0 / float(b_total)
    hb = sl // 2

    # flush blocks: bank k covers chunks [k*NB, min((k+1)*NB, nchunk))
    nflush = (nchunk + NB - 1) // NB
    nreg = 2 * nflush

    nc = bacc.Bacc("TRN2", target_bir_lowering=False, debug=False,
                   num_devices=N_CORES)

    def din(name, shape, dt):
        return nc.dram_tensor(name, shape, dt, kind="ExternalInput").ap()

    xb = din("xb", [2 * C_IN, p_pad // 2], BF16)
    s_in = din("S", [128, nchunk * 4 * 12], BF16)
    xsT = din("xsT", [S_SCALAR, sl], BF16)
    # bf16 const blob: w2s cols 0:128, w1s cols 128:256 (rows 0:32),
    # mw1t cols 256:384 (rows 0:72), mw2t cols 384:448, mw3t col 448
    cb = din("cb", [128, 449], BF16)
    # f32 const blob: bn1_g, bn1_b, bn2_g, bn2_b, b3 (col 4, row 0)
    cf = din("cf", [128, 5], F32)

    y_out = nc.dram_tensor("y", [1, sl], F32, kind="ExternalOutput").ap()

    XCHUNK = 16
    SCHUNK = 16

    def col_tiles():
        n512 = (sl + 511) // 512
        for i in range(n512):
            yield i * 512, min(sl, (i + 1) * 512)

    with tile.TileContext(nc) as tc, ExitStack() as ctx:
        const_pool = ctx.enter_context(tc.tile_pool(name="const", bufs=1))
        xb_pool = ctx.enter_context(tc.tile_pool(name="xb", bufs=2))
        s_pool = ctx.enter_context(tc.tile_pool(name="spool", bufs=2))
        h1_pool = ctx.enter_context(tc.tile_pool(name="h1", bufs=4))
        h2_pool = ctx.enter_context(tc.tile_pool(name="h2", bufs=4))
        z_pool = ctx.enter_context(tc.tile_pool(name="z", bufs=1))
        mlp_pool = ctx.enter_context(tc.tile_pool(name="mlp", bufs=1))
        stat_pool = ctx.enter_context(tc.tile_pool(name="stat", bufs=1))
        ps_a = ctx.enter_context(tc.tile_pool(name="psa", bufs=3, space="PSUM"))
        ps_b = ctx.enter_context(tc.tile_pool(name="psb", bufs=3, space="PSUM"))
        ps_z = ctx.enter_context(tc.tile_pool(name="psz", bufs=2, space="PSUM"))
        dram = ctx.enter_context(tc.tile_pool(name="dram", bufs=1, space="DRAM"))

        # ---------- startup: data DMAs first, then consts, then CC warmup ----
        xb_t = xb_pool.tile([2 * C_IN, XCHUNK * 512], BF16, tag="xb")
        nc.sync.dma_start(xb_t[:], xb[:, 0:XCHUNK * 512])
        s_t = s_pool.tile([128, SCHUNK * 48], BF16, tag="st")
        nc.sync.dma_start(s_t[:], s_in[:, 0:SCHUNK * 48])

        cb_s = const_pool.tile([128, 449], BF16)
        nc.scalar.dma_start(cb_s[:], cb[:])
        cf_s = const_pool.tile([128, 5], F32)
        nc.scalar.dma_start(cf_s[:], cf[:])

        z_s = z_pool.tile([F + S_SCALAR, sl], BF16)
        nc.scalar.dma_start(z_s[F:F + S_SCALAR, :], xsT[:])

        w2s_s = cb_s[:, 0:128]
        w1s_s = cb_s[0:2 * C_IN, 128:256]
        mw1t_s = cb_s[0:F + S_SCALAR, 256:384]
        mw2t_s = cb_s[:, 384:448]
        mw3t_s = cb_s[0:M2, 448:449]
        g1_s = cf_s[:, 0:1]
        bb1_s = cf_s[:, 1:2]
        g2_s = cf_s[0:M2, 2:3]
        bb2_s = cf_s[0:M2, 3:4]
        b3_s = cf_s[0:1, 4:5]

        # warm up the collective stream so the real AllReduces are cheap
        warm_in = dram.tile([1, 1], F32)
        warm_out = dram.tile([1, 1], F32)
        wsrc = const_pool.tile([1, 1], F32)
        nc.vector.memset(wsrc[:], 0.0)
        nc.gpsimd.dma_start(warm_in[:], wsrc[:])
        nc.gpsimd.collective_compute(
            "AllReduce", OP.add, replica_groups=[list(range(N_CORES))],
            ins=[warm_in.opt()], outs=[warm_out.opt()])

        # v1 partial stats: per flush-region sums / sum-of-squares
        sp_sum = stat_pool.tile([M1, nreg], F32, tag="spsum")
        sp_sq = stat_pool.tile([M1, nreg], F32, tag="spsq")
        v1_s = mlp_pool.tile([M1, sl], F32, tag="v1")

        # ---------- main loop ----------
        zp = None
        for j in range(nchunk):
            if j % XCHUNK == 0 and j > 0:
                nx = min(XCHUNK, nchunk - j)
                xb_t = xb_pool.tile([2 * C_IN, XCHUNK * 512], BF16, tag="xb")
                nc.sync.dma_start(xb_t[:, :nx * 512],
                                  xb[:, j * 512:(j + nx) * 512])
            if j % SCHUNK == 0 and j > 0:
                ns = min(SCHUNK, nchunk - j)
                s_t = s_pool.tile([128, SCHUNK * 48], BF16, tag="st")
                nc.sync.dma_start(s_t[:, :ns * 48],
                                  s_in[:, j * 48:(j + ns) * 48])
            jx = (j % XCHUNK) * 512
            js = (j % SCHUNK) * 48

            p1 = ps_a.tile([128, 512], F32, tag="p1")
            nc.tensor.matmul(p1[:, :], w1s_s, xb_t[:, jx:jx + 512],
                             start=True, stop=True)
            h1_t = h1_pool.tile([128, 512], BF16, tag="h1")
            if j % 2 == 0:
                nc.vector.tensor_scalar(h1_t[:], p1[:], 0.0, None, OP.max)
            else:
                nc.scalar.activation(h1_t[:], p1[:], ACTF.Relu)

            # block-diagonal W2 computes the A-half (cols 0:64) and B-half
            # (cols 64:128) h2^T of each k-tile in one matmul
            p2 = ps_b.tile([128, 512], F32, tag="p2")
            for t in range(4):
                nc.tensor.matmul(p2[:, 128 * t:128 * t + 128],
                                 h1_t[:, 128 * t:128 * t + 128], w2s_s,
                                 start=True, stop=True)
            h2_t = h2_pool.tile([128, 512], BF16, tag="h2")
            if j % 2 == 1:
                nc.vector.tensor_scalar(h2_t[:], p2[:], 0.0, None, OP.max)
            else:
                nc.scalar.activation(h2_t[:], p2[:], ACTF.Relu)

            # pool: one matmul per k-tile; 12 S cols = 6 A-slots (valid at
            # PSUM rows 0:64) + 6 B-slots (valid rows 64:128); garbage
            # quadrants are never evicted.
            if j % NB == 0:
                zp = ps_z.tile([128, 504], F32, tag="zp")
            zc = (j % NB) * 12
            for t in range(4):
                nc.tensor.matmul(
                    zp[:, zc:zc + 12], h2_t[:, 128 * t:128 * t + 128],
                    s_t[:, js + 12 * t: js + 12 * t + 12],
                    start=(t == 0), stop=(t == 3))

            if (j + 1) % NB == 0 or j == nchunk - 1:
                k = j // NB
                j0 = k * NB
                nb = j - j0 + 1
                zv = zp.rearrange("p (b c) -> p b c", c=12)
                for h, (r0, coff) in enumerate(((0, 0), (64, hb))):
                    c0 = coff + 6 * j0
                    dst = z_s[0:F, c0:c0 + 6 * nb].rearrange(
                        "p (b c) -> p b c", c=6)
                    nc.vector.tensor_copy(
                        dst, zv[r0:r0 + F, 0:nb, 6 * h:6 * h + 6])
                    # pipelined v1 + stats for this flushed region
                    pv = ps_b.tile([128, 512], F32, tag="p2")
                    nc.tensor.matmul(pv[:, :6 * nb], mw1t_s,
                                     z_s[0:F + S_SCALAR, c0:c0 + 6 * nb],
                                     start=True, stop=True)
                    ridx = 2 * k + h
                    nc.scalar.activation(v1_s[:, c0:c0 + 6 * nb],
                                         pv[:, :6 * nb], ACTF.Copy,
                                         accum_out=sp_sum[:, ridx:ridx + 1])
                    scr = mlp_pool.tile([M1, 504], F32, tag="scr", bufs=2)
                    nc.scalar.activation(scr[:, :6 * nb], pv[:, :6 * nb],
                                         ACTF.Square,
                                         accum_out=sp_sq[:, ridx:ridx + 1])

        # ---------- BN1 ----------
        s1 = stat_pool.tile([M1, 2], F32, tag="s1")
        nc.vector.tensor_reduce(s1[:, 0:1], sp_sum[:], axis=AX.X, op=OP.add)
        nc.vector.tensor_reduce(s1[:, 1:2], sp_sq[:], axis=AX.X, op=OP.add)
        cc_in1 = dram.tile([M1, 2], F32)
        cc_out1 = dram.tile([M1, 2], F32)
        nc.sync.dma_start(cc_in1[:], s1[:])
        nc.gpsimd.collective_compute(
            "AllReduce", OP.add, replica_groups=[list(range(N_CORES))],
            ins=[cc_in1.opt()], outs=[cc_out1.opt()])
        ar1 = stat_pool.tile([M1, 2], F32, tag="ar1")
        nc.sync.dma_start(ar1[:], cc_out1[:])

        t1 = stat_pool.tile([M1, 8], F32, tag="t1")
        mean1, m21, var1, inv1, rstd1, sc1, ms1, of1 = (
            t1[:, i:i + 1] for i in range(8))
        nc.vector.tensor_scalar(mean1, ar1[:, 0:1], inv_b, None, OP.mult)
        nc.vector.tensor_tensor(m21, mean1, mean1, OP.mult)
        nc.vector.tensor_scalar(m21, m21, EPS, None, OP.subtract)
        nc.vector.scalar_tensor_tensor(
            var1, ar1[:, 1:2], inv_b, m21, OP.mult, OP.subtract)
        nc.vector.reciprocal(inv1, var1)
        nc.scalar.activation(rstd1, inv1, ACTF.Sqrt)
        nc.vector.tensor_tensor(sc1, g1_s, rstd1, OP.mult)
        nc.vector.tensor_tensor(ms1, mean1, sc1, OP.mult)
        nc.vector.tensor_tensor(of1, bb1_s, ms1, OP.subtract)

        # ---------- layer 2, tiled so matmuls start as a1 tiles complete ----
        a1_s = mlp_pool.tile([M1, sl], BF16, tag="v1a")
        v2_s = mlp_pool.tile([M2, sl], F32, tag="v2")
        sp2_sum = stat_pool.tile([M2, 4], F32, tag="sp2sum")
        sp2_sq = stat_pool.tile([M2, 4], F32, tag="sp2sq")
        for k, (a, b) in enumerate(col_tiles()):
            nc.scalar.activation(a1_s[:, a:b], v1_s[:, a:b], ACTF.Relu,
                                 bias=of1, scale=sc1)
            pv = ps_a.tile([128, 512], F32, tag="p1")
            nc.tensor.matmul(pv[0:M2, :b - a], mw2t_s, a1_s[:, a:b],
                             start=True, stop=True)
            nc.vector.tensor_copy(v2_s[:, a:b], pv[0:M2, :b - a])
            nc.vector.tensor_reduce(sp2_sum[:, k:k + 1], pv[0:M2, :b - a],
                                    axis=AX.X, op=OP.add)
            scr2 = mlp_pool.tile([M2, 512], F32, tag="scr2", bufs=2)
            nc.vector.tensor_tensor(scr2[:, :b - a], pv[0:M2, :b - a],
                                    v2_s[:, a:b], OP.mult)
            nc.vector.tensor_reduce(sp2_sq[:, k:k + 1], scr2[:, :b - a],
                                    axis=AX.X, op=OP.add)
        s2 = stat_pool.tile([M2, 2], F32, tag="s2")
        nc.vector.tensor_reduce(s2[:, 0:1], sp2_sum[:], axis=AX.X, op=OP.add)
        nc.vector.tensor_reduce(s2[:, 1:2], sp2_sq[:], axis=AX.X, op=OP.add)
        cc_in2 = dram.tile([M2, 2], F32)
        cc_out2 = dram.tile([M2, 2], F32)
        nc.sync.dma_start(cc_in2[:], s2[:])
        nc.gpsimd.collective_compute(
            "AllReduce", OP.add, replica_groups=[list(range(N_CORES))],
            ins=[cc_in2.opt()], outs=[cc_out2.opt()])

        # empty-slot correction, computed while the AllReduce is in flight:
        # a1_empty = relu(of1); ve = W2 @ a1_empty
        a1e = stat_pool.tile([M1, 1], BF16, tag="a1e")
        nc.scalar.activation(a1e[:], of1, ACTF.Relu)
        pve = ps_b.tile([128, 512], F32, tag="p2")
        nc.tensor.matmul(pve[0:M2, 0:1], mw2t_s, a1e[:], start=True, stop=True)
        ve = stat_pool.tile([M2, 2], F32, tag="ve")
        nc.vector.tensor_copy(ve[:, 0:1], pve[0:M2, 0:1])
        nc.vector.tensor_tensor(ve[:, 1:2], ve[:, 0:1], ve[:, 0:1], OP.mult)

        ar2 = stat_pool.tile([M2, 2], F32, tag="ar2")
        nc.sync.dma_start(ar2[:], cc_out2[:])
        s2c = stat_pool.tile([M2, 2], F32, tag="s2c")
        nc.vector.scalar_tensor_tensor(
            s2c[:, 0:1], ve[:, 0:1], -n_empty, ar2[:, 0:1], OP.mult, OP.add)
        nc.vector.scalar_tensor_tensor(
            s2c[:, 1:2], ve[:, 1:2], -n_empty, ar2[:, 1:2], OP.mult, OP.add)

        t2 = stat_pool.tile([M2, 8], F32, tag="t2")
        mean2, m22, var2, inv2, rstd2, sc2, ms2, of2 = (
            t2[:, i:i + 1] for i in range(8))
        nc.vector.tensor_scalar(mean2, s2c[:, 0:1], inv_b, None, OP.mult)
        nc.vector.tensor_tensor(m22, mean2, mean2, OP.mult)
        nc.vector.tensor_scalar(m22, m22, EPS, None, OP.subtract)
        nc.vector.scalar_tensor_tensor(
            var2, s2c[:, 1:2], inv_b, m22, OP.mult, OP.subtract)
        nc.vector.reciprocal(inv2, var2)
        nc.scalar.activation(rstd2, inv2, ACTF.Sqrt)
        nc.vector.tensor_tensor(sc2, g2_s, rstd2, OP.mult)
        nc.vector.tensor_tensor(ms2, mean2, sc2, OP.mult)
        nc.vector.tensor_tensor(of2, bb2_s, ms2, OP.subtract)

        # raw logits only; bias b3 + sigmoid are applied on the host
        a2_s = mlp_pool.tile([M2, sl], BF16, tag="v2a")
        y_s = mlp_pool.tile([1, sl], F32, tag="y")
        for a, b in col_tiles():
            nc.scalar.activation(a2_s[:, a:b], v2_s[:, a:b], ACTF.Relu,
                                 bias=of2, scale=sc2)
            pv = ps_a.tile([128, 512], F32, tag="p1")
            nc.tensor.matmul(pv[0:1, :b - a], mw3t_s, a2_s[:, a:b],
                             start=True, stop=True)
            nc.vector.tensor_copy(y_s[:, a:b], pv[0:1, :b - a])
        nc.sync.dma_start(y_out[:], y_s[:])

    nc.compile()
    return nc


_CACHE = {}


def kernel(**inputs) -> np.ndarray:
    x = np.asarray(inputs["x_set"], np.float32)[0]        # [16, T]
    x_scalar = np.asarray(inputs["x_scalar"], np.float32)  # [B, 8]
    lengths = np.asarray(inputs["sample_indices"])[0].astype(np.int64)
    b_total = x_scalar.shape[0]
    offsets = np.concatenate([[0], np.cumsum(lengths)[:-1]])

    groups, ng = _plan_groups(lengths, b_total)
    sl = ng * SLOTS
    per_core = [
        _prep_core(x, x_scalar, lengths, offsets, groups[c], ng)
        for c in range(N_CORES)
    ]

    b1 = np.asarray(inputs["phi_b1"], np.float32)
    b2 = np.asarray(inputs["phi_b2"], np.float32)
    assert np.all(b1 == 0.0) and np.all(b2 == 0.0), \
        "nonzero phi bias path not implemented"

    cb = np.zeros((128, 449), dtype=np.float32)
    w1t = np.asarray(inputs["phi_w1"], np.float32).T      # [16, 64]
    w2t = np.asarray(inputs["phi_w2"], np.float32).T      # [64, 64]
    cb[0:F, 0:F] = w2t
    cb[F:128, F:128] = w2t
    cb[0:C_IN, 128:128 + F] = w1t
    cb[C_IN:2 * C_IN, 128 + F:256] = w1t
    cb[0:F + S_SCALAR, 256:384] = np.asarray(inputs["mlp_w1"], np.float32).T
    cb[0:M1, 384:448] = np.asarray(inputs["mlp_w2"], np.float32).T
    cb[0:M2, 448:449] = np.asarray(inputs["mlp_w3"], np.float32).T

    cf = np.zeros((128, 5), dtype=np.float32)
    cf[:, 0] = np.asarray(inputs["bn1_g"], np.float32)
    cf[:, 1] = np.asarray(inputs["bn1_b"], np.float32)
    cf[0:M2, 2] = np.asarray(inputs["bn2_g"], np.float32)
    cf[0:M2, 3] = np.asarray(inputs["bn2_b"], np.float32)
    cf[0, 4] = float(np.asarray(inputs["mlp_b3"], np.float32).reshape(()))

    consts = {
        "cb": np.ascontiguousarray(cb.astype(ml_dtypes.bfloat16)),
        "cf": np.ascontiguousarray(cf),
    }

    key = (ng, sl, b_total)
    if key not in _CACHE:
        _CACHE[key] = _build_nc(ng, sl, b_total)
    nc = _CACHE[key]

    in_maps = []
    for pc in per_core:
        m = {"xb": pc["xb"], "S": pc["S"], "xsT": pc["xsT"]}
        m.update(consts)
        in_maps.append(m)

    res = bass_utils.run_bass_kernel_spmd(
        nc, in_maps, core_ids=list(range(N_CORES)))

    b3 = float(np.asarray(inputs["mlp_b3"], np.float32).reshape(()))
    y = np.zeros((b_total, 1), dtype=np.float32)
    for c, pc in enumerate(per_core):
        ys = res.results[c]["y"][0]
        se = pc["slot_events"]
        mask = se >= 0
        y[se[mask], 0] = ys[mask]
    y = 1.0 / (1.0 + np.exp(-(y + b3)))
    return y.astype(np.float32)


# revision 27
# speedup vs baseline: 1.1217x; 1.1217x over previous
"""DeepSets ensemble (segment mean-pool + BN MLP) on 8 TRN2 NeuronCores.

Strategy (data-parallel, per sharding hint):
 - events are split 1024/core; each core's points are bin-packed (FFD) into
   512-pt groups of whole events (<=6 per group), zero-padded, so the ragged
   segment-sum becomes a block matmul against a host-built selector S whose
   entries are 1/len (mean pooling folded into the matmul).
 - per core: phi1 (x [16,512]-tiles bf16 -> PSUM), fused-relu PSUM->SBUF
   copies (alternating DVE/ACT), phi2 with the h1-tile as the stationary
   matmul operand producing h2^T [128pts, 64] tiles, then pooling matmuls
   h2^T x S (12 merged A/B slot cols per k-tile) accumulating z in PSUM.
 - MLP sharded over slots, computed in bf16; BatchNorm uses two tiny
   AllReduces of (sum, sum-of-squares); a dummy AllReduce at kernel start
   warms the collective stream; v1 + its stats are pipelined into the main
   loop per flushed PSUM bank; empty-slot contributions are corrected
   analytically after the second AllReduce.
 - host scatters x_scalar / gathers y through the slot<->event map.
"""
import sys
import numpy as np
import ml_dtypes
from contextlib import ExitStack

sys.path.insert(0, "/opt/trn_rl_repo")

import concourse.bacc as bacc
import concourse.tile as tile
from concourse import mybir
from concourse import bass_utils

BF16 = mybir.dt.bfloat16
F32 = mybir.dt.float32
AX = mybir.AxisListType
OP = mybir.AluOpType
ACTF = mybir.ActivationFunctionType

N_CORES = 8
C_IN = 16
F = 64
S_SCALAR = 8
M1, M2 = 128, 64
G = 512
SLOTS = 6
NB = 42          # chunks per pooling PSUM bank (12 cols each -> 504)
EPS = 1e-5


def _plan_groups(lengths, b_total):
    e_per_core = b_total // N_CORES
    cores = []
    for c in range(N_CORES):
        evs = sorted(range(c * e_per_core, (c + 1) * e_per_core),
                     key=lambda e: -int(lengths[e]))
        groups, space = [], []
        for e in evs:
            l = int(lengths[e])
            assert 0 < l <= G
            placed = False
            for gi in range(len(groups)):
                if space[gi] >= l and len(groups[gi]) < SLOTS:
                    groups[gi].append(e)
                    space[gi] -= l
                    placed = True
                    break
            if not placed:
                groups.append([e])
                space.append(G - l)
        cores.append(groups)
    ng = max(len(g) for g in cores)
    if ng % 2:
        ng += 1
    for g in cores:
        while len(g) < ng:
            g.append([])
    return cores, ng


def _prep_core(x, x_scalar, lengths, offsets, groups, ng):
    nchunk = ng // 2
    p_pad = ng * G
    sl = ng * SLOTS
    xb = np.zeros((2 * C_IN, p_pad // 2), dtype=np.float32)
    # merged A/B selector: per chunk j, 4 k-tile blocks of 12 cols
    # (6 A-slot cols then 6 B-slot cols), entries are 1/len
    s_mat = np.zeros((128, nchunk * 4 * 12), dtype=np.float32)
    xsT = np.zeros((S_SCALAR, sl), dtype=np.float32)
    slot_events = np.full(sl, -1, dtype=np.int64)
    for j in range(nchunk):
        for half, g_idx in ((0, j), (1, nchunk + j)):
            evs = groups[g_idx]
            col0 = 512 * j
            row0 = C_IN * half
            pt = 0
            for i, e in enumerate(evs):
                l = int(lengths[e])
                o = int(offsets[e])
                xb[row0:row0 + C_IN, col0 + pt: col0 + pt + l] = x[:, o:o + l]
                p_arr = np.arange(pt, pt + l)
                s_mat[p_arr % 128,
                      (4 * j + p_arr // 128) * 12 + 6 * half + i] = 1.0 / l
                slot = SLOTS * g_idx + i
                xsT[:, slot] = x_scalar[e]
                slot_events[slot] = e
                pt += l
    return {
        "xb": np.ascontiguousarray(xb.astype(ml_dtypes.bfloat16)),
        "S": np.ascontiguousarray(s_mat.astype(ml_dtypes.bfloat16)),
        "xsT": np.ascontiguousarray(xsT.astype(ml_dtypes.bfloat16)),
        "slot_events": slot_events,
    }


def _build_nc(ng, sl, b_total):
    nchunk = ng // 2
    p_pad = ng * G
    n_empty = float(N_CORES * sl - b_total)
    inv_b = 1.0 / float(b_total)
    hb = sl // 2

    # flush blocks: bank k covers chunks [k*NB, min((k+1)*NB, nchunk))
    nflush = (nchunk + NB - 1) // NB
    nreg = 2 * nflush

    nc = bacc.Bacc("TRN2", target_bir_lowering=False, debug=False,
                   num_devices=N_CORES)

    def din(name, shape, dt):
        return nc.dram_tensor(name, shape, dt, kind="ExternalInput").ap()

    xb = din("xb", [2 * C_IN, p_pad // 2], BF16)
    s_in = din("S", [128, nchunk * 4 * 12], BF16)
    xsT = din("xsT", [S_SCALAR, sl], BF16)
    # bf16 const blob: w2s cols 0:128, w1s cols 128:256 (rows 0:32),
    # mw1t cols 256:384 (rows 0:72), mw2t cols 384:448, mw3t col 448
    cb = din("cb", [128, 449], BF16)
    # f32 const blob: bn1_g, bn1_b, bn2_g, bn2_b, b3 (col 4, row 0)
    cf = din("cf", [128, 5], F32)

    y_out = nc.dram_tensor("y", [1, sl], F32, kind="ExternalOutput").ap()

    XCHUNK = 16
    SCHUNK = 16

    def col_tiles():
        n512 = (sl + 511) // 512
        for i in range(n512):
            yield i * 512, min(sl, (i + 1) * 512)

    with tile.TileContext(nc) as tc, ExitStack() as ctx:
        const_pool = ctx.enter_context(tc.tile_pool(name="const", bufs=1))
        xb_pool = ctx.enter_context(tc.tile_pool(name="xb", bufs=2))
        s_pool = ctx.enter_context(tc.tile_pool(name="spool", bufs=2))
        h1_pool = ctx.enter_context(tc.tile_pool(name="h1", bufs=4))
        h2_pool = ctx.enter_context(tc.tile_pool(name="h2", bufs=4))
        z_pool = ctx.enter_context(tc.tile_pool(name="z", bufs=1))
        mlp_pool = ctx.enter_context(tc.tile_pool(name="mlp", bufs=1))
        stat_pool = ctx.enter_context(tc.tile_pool(name="stat", bufs=1))
        ps_a = ctx.enter_context(tc.tile_pool(name="psa", bufs=3, space="PSUM"))
        ps_b = ctx.enter_context(tc.tile_pool(name="psb", bufs=3, space="PSUM"))
        ps_z = ctx.enter_context(tc.tile_pool(name="psz", bufs=2, space="PSUM"))
        dram = ctx.enter_context(tc.tile_pool(name="dram", bufs=1, space="DRAM"))

        # ---------- startup: data DMAs first, then consts, then CC warmup ----
        xb_t = xb_pool.tile([2 * C_IN, XCHUNK * 512], BF16, tag="xb")
        nc.sync.dma_start(xb_t[:], xb[:, 0:XCHUNK * 512])
        s_t = s_pool.tile([128, SCHUNK * 48], BF16, tag="st")
        nc.sync.dma_start(s_t[:], s_in[:, 0:SCHUNK * 48])

        cb_s = const_pool.tile([128, 449], BF16)
        nc.scalar.dma_start(cb_s[:], cb[:])
        cf_s = const_pool.tile([128, 5], F32)
        nc.scalar.dma_start(cf_s[:], cf[:])

        z_s = z_pool.tile([F + S_SCALAR, sl], BF16)
        nc.scalar.dma_start(z_s[F:F + S_SCALAR, :], xsT[:])

        w2s_s = cb_s[:, 0:128]
        w1s_s = cb_s[0:2 * C_IN, 128:256]
        mw1t_s = cb_s[0:F + S_SCALAR, 256:384]
        mw2t_s = cb_s[:, 384:448]
        mw3t_s = cb_s[0:M2, 448:449]
        g1_s = cf_s[:, 0:1]
        bb1_s = cf_s[:, 1:2]
        g2_s = cf_s[0:M2, 2:3]
        bb2_s = cf_s[0:M2, 3:4]
        b3_s = cf_s[0:1, 4:5]

        # warm up the collective stream so the real AllReduces are cheap
        warm_in = dram.tile([1, 1], F32)
        warm_out = dram.tile([1, 1], F32)
        wsrc = const_pool.tile([1, 1], F32)
        nc.vector.memset(wsrc[:], 0.0)
        nc.gpsimd.dma_start(warm_in[:], wsrc[:])
        nc.gpsimd.collective_compute(
            "AllReduce", OP.add, replica_groups=[list(range(N_CORES))],
            ins=[warm_in.opt()], outs=[warm_out.opt()])

        # v1 partial stats: per flush-region sums / sum-of-squares
        sp_sum = stat_pool.tile([M1, nreg], F32, tag="spsum")
        sp_sq = stat_pool.tile([M1, nreg], F32, tag="spsq")
        v1_s = mlp_pool.tile([M1, sl], F32, tag="v1")

        # ---------- main loop, software-pipelined ----------
        # iteration j issues phi1(j), phi2(j-1), pool(j-2), v1-stage for a
        # bank flushed at j-2: every tensor instruction's inputs were
        # produced >=1 chunk earlier, keeping the PE stream free of
        # dependency stalls (which would reset its p-state ramp).
        h1s, h2s, zps, sts = {}, {}, {}, {}
        v1_pend = []

        def do_v1_region(k, h, nb):
            j0 = k * NB
            c0 = (hb if h else 0) + 6 * j0
            pv = ps_b.tile([128, 512], F32, tag="p2")
            nc.tensor.matmul(pv[:, :6 * nb], mw1t_s,
                             z_s[0:F + S_SCALAR, c0:c0 + 6 * nb],
                             start=True, stop=True)
            ridx = 2 * k + h
            nc.scalar.activation(v1_s[:, c0:c0 + 6 * nb],
                                 pv[:, :6 * nb], ACTF.Copy,
                                 accum_out=sp_sum[:, ridx:ridx + 1])
            scr = mlp_pool.tile([M1, 504], F32, tag="scr", bufs=2)
            nc.scalar.activation(scr[:, :6 * nb], pv[:, :6 * nb],
                                 ACTF.Square,
                                 accum_out=sp_sq[:, ridx:ridx + 1])

        sts[0] = s_t
        xbs = {0: xb_t}
        for j in range(nchunk + 2):
            if j < nchunk:
                if j % XCHUNK == 8 and j + 8 < nchunk:
                    jn = (j // XCHUNK + 1) * XCHUNK
                    nx = min(XCHUNK, nchunk - jn)
                    xb_n = xb_pool.tile([2 * C_IN, XCHUNK * 512], BF16,
                                        tag="xb")
                    nc.sync.dma_start(xb_n[:, :nx * 512],
                                      xb[:, jn * 512:(jn + nx) * 512])
                    xbs[jn // XCHUNK] = xb_n
                if j % SCHUNK == 8 and j + 8 < nchunk:
                    jn = (j // SCHUNK + 1) * SCHUNK
                    ns = min(SCHUNK, nchunk - jn)
                    s_n = s_pool.tile([128, SCHUNK * 48], BF16, tag="st")
                    nc.sync.dma_start(s_n[:, :ns * 48],
                                      s_in[:, jn * 48:(jn + ns) * 48])
                    sts[jn // SCHUNK] = s_n
                jx = (j % XCHUNK) * 512
                p1 = ps_a.tile([128, 512], F32, tag="p1")
                nc.tensor.matmul(p1[:, :], w1s_s,
                                 xbs[j // XCHUNK][:, jx:jx + 512],
                                 start=True, stop=True)
                h1_t = h1_pool.tile([128, 512], BF16, tag="h1")
                if j % 2 == 0:
                    nc.vector.tensor_scalar(h1_t[:], p1[:], 0.0, None, OP.max)
                else:
                    nc.scalar.activation(h1_t[:], p1[:], ACTF.Relu)
                h1s[j] = h1_t

            if 1 <= j <= nchunk:
                # block-diagonal W2 computes the A-half (cols 0:64) and
                # B-half (cols 64:128) h2^T of each k-tile in one matmul
                jj = j - 1
                h1_t = h1s.pop(jj)
                p2 = ps_b.tile([128, 512], F32, tag="p2")
                for t in range(4):
                    nc.tensor.matmul(p2[:, 128 * t:128 * t + 128],
                                     h1_t[:, 128 * t:128 * t + 128], w2s_s,
                                     start=True, stop=True)
                h2_t = h2_pool.tile([128, 512], BF16, tag="h2")
                if jj % 2 == 1:
                    nc.vector.tensor_scalar(h2_t[:], p2[:], 0.0, None, OP.max)
                else:
                    nc.scalar.activation(h2_t[:], p2[:], ACTF.Relu)
                h2s[jj] = h2_t

            if v1_pend:
                do_v1_region(*v1_pend.pop(0))

            if j >= 2:
                # pool: one matmul per k-tile; 12 S cols = 6 A-slots (valid
                # at PSUM rows 0:64) + 6 B-slots (valid rows 64:128);
                # garbage quadrants are never evicted.
                jj = j - 2
                h2_t = h2s.pop(jj)
                if jj % NB == 0:
                    zp = ps_z.tile([128, 504], F32, tag="zp")
                zc = (jj % NB) * 12
                js = (jj % SCHUNK) * 48
                s_t = sts[jj // SCHUNK]
                for t in range(4):
                    nc.tensor.matmul(
                        zp[:, zc:zc + 12], h2_t[:, 128 * t:128 * t + 128],
                        s_t[:, js + 12 * t: js + 12 * t + 12],
                        start=(t == 0), stop=(t == 3))

                if (jj + 1) % NB == 0 or jj == nchunk - 1:
                    k = jj // NB
                    nb = jj - k * NB + 1
                    zv = zp.rearrange("p (b c) -> p b c", c=12)
                    for h, (r0, coff) in enumerate(((0, 0), (64, hb))):
                        c0 = coff + 6 * k * NB
                        dst = z_s[0:F, c0:c0 + 6 * nb].rearrange(
                            "p (b c) -> p b c", c=6)
                        nc.vector.tensor_copy(
                            dst, zv[r0:r0 + F, 0:nb, 6 * h:6 * h + 6])
                        v1_pend.append((k, h, nb))

        while v1_pend:
            do_v1_region(*v1_pend.pop(0))

        # ---------- BN1 ----------
        s1 = stat_pool.tile([M1, 2], F32, tag="s1")
        nc.vector.tensor_reduce(s1[:, 0:1], sp_sum[:], axis=AX.X, op=OP.add)
        nc.vector.tensor_reduce(s1[:, 1:2], sp_sq[:], axis=AX.X, op=OP.add)
        cc_in1 = dram.tile([M1, 2], F32)
        cc_out1 = dram.tile([M1, 2], F32)
        nc.sync.dma_start(cc_in1[:], s1[:])
        nc.gpsimd.collective_compute(
            "AllReduce", OP.add, replica_groups=[list(range(N_CORES))],
            ins=[cc_in1.opt()], outs=[cc_out1.opt()])
        ar1 = stat_pool.tile([M1, 2], F32, tag="ar1")
        nc.sync.dma_start(ar1[:], cc_out1[:])

        t1 = stat_pool.tile([M1, 8], F32, tag="t1")
        mean1, m21, var1, inv1, rstd1, sc1, ms1, of1 = (
            t1[:, i:i + 1] for i in range(8))
        nc.vector.tensor_scalar(mean1, ar1[:, 0:1], inv_b, None, OP.mult)
        nc.vector.tensor_tensor(m21, mean1, mean1, OP.mult)
        nc.vector.tensor_scalar(m21, m21, EPS, None, OP.subtract)
        nc.vector.scalar_tensor_tensor(
            var1, ar1[:, 1:2], inv_b, m21, OP.mult, OP.subtract)
        nc.vector.reciprocal(inv1, var1)
        nc.scalar.activation(rstd1, inv1, ACTF.Sqrt)
        nc.vector.tensor_tensor(sc1, g1_s, rstd1, OP.mult)
        nc.vector.tensor_tensor(ms1, mean1, sc1, OP.mult)
        nc.vector.tensor_tensor(of1, bb1_s, ms1, OP.subtract)

        # ---------- layer 2, tiled so matmuls start as a1 tiles complete ----
        a1_s = mlp_pool.tile([M1, sl], BF16, tag="v1a")
        v2_s = mlp_pool.tile([M2, sl], F32, tag="v2")
        sp2_sum = stat_pool.tile([M2, 4], F32, tag="sp2sum")
        sp2_sq = stat_pool.tile([M2, 4], F32, tag="sp2sq")
        for k, (a, b) in enumerate(col_tiles()):
            nc.scalar.activation(a1_s[:, a:b], v1_s[:, a:b], ACTF.Relu,
                                 bias=of1, scale=sc1)
            pv = ps_a.tile([128, 512], F32, tag="p1")
            nc.tensor.matmul(pv[0:M2, :b - a], mw2t_s, a1_s[:, a:b],
                             start=True, stop=True)
            nc.vector.tensor_copy(v2_s[:, a:b], pv[0:M2, :b - a])
            nc.vector.tensor_reduce(sp2_sum[:, k:k + 1], pv[0:M2, :b - a],
                                    axis=AX.X, op=OP.add)
            scr2 = mlp_pool.tile([M2, 512], F32, tag="scr2", bufs=2)
            nc.vector.tensor_tensor(scr2[:, :b - a], pv[0:M2, :b - a],
                                    v2_s[:, a:b], OP.mult)
            nc.vector.tensor_reduce(sp2_sq[:, k:k + 1], scr2[:, :b - a],
                                    axis=AX.X, op=OP.add)
        s2 = stat_pool.tile([M2, 2], F32, tag="s2")
        nc.vector.tensor_reduce(s2[:, 0:1], sp2_sum[:], axis=AX.X, op=OP.add)
        nc.vector.tensor_reduce(s2[:, 1:2], sp2_sq[:], axis=AX.X, op=OP.add)
        cc_in2 = dram.tile([M2, 2], F32)
        cc_out2 = dram.tile([M2, 2], F32)
        nc.sync.dma_start(cc_in2[:], s2[:])
        nc.gpsimd.collective_compute(
            "AllReduce", OP.add, replica_groups=[list(range(N_CORES))],
            ins=[cc_in2.opt()], outs=[cc_out2.opt()])

        # empty-slot correction, computed while the AllReduce is in flight:
        # a1_empty = relu(of1); ve = W2 @ a1_empty
        a1e = stat_pool.tile([M1, 1], BF16, tag="a1e")
        nc.scalar.activation(a1e[:], of1, ACTF.Relu)
        pve = ps_b.tile([128, 512], F32, tag="p2")
        nc.tensor.matmul(pve[0:M2, 0:1], mw2t_s, a1e[:], start=True, stop=True)
        ve = stat_pool.tile([M2, 2], F32, tag="ve")
        nc.vector.tensor_copy(ve[:, 0:1], pve[0:M2, 0:1])
        nc.vector.tensor_tensor(ve[:, 1:2], ve[:, 0:1], ve[:, 0:1], OP.mult)

        ar2 = stat_pool.tile([M2, 2], F32, tag="ar2")
        nc.sync.dma_start(ar2[:], cc_out2[:])
        s2c = stat_pool.tile([M2, 2], F32, tag="s2c")
        nc.vector.scalar_tensor_tensor(
            s2c[:, 0:1], ve[:, 0:1], -n_empty, ar2[:, 0:1], OP.mult, OP.add)
        nc.vector.scalar_tensor_tensor(
            s2c[:, 1:2], ve[:, 1:2], -n_empty, ar2[:, 1:2], OP.mult, OP.add)

        t2 = stat_pool.tile([M2, 8], F32, tag="t2")
        mean2, m22, var2, inv2, rstd2, sc2, ms2, of2 = (
            t2[:, i:i + 1] for i in range(8))
        nc.vector.tensor_scalar(mean2, s2c[:, 0:1], inv_b, None, OP.mult)
        nc.vector.tensor_tensor(m22, mean2, mean2, OP.mult)
        nc.vector.tensor_scalar(m22, m22, EPS, None, OP.subtract)
        nc.vector.scalar_tensor_tensor(
            var2, s2c[:, 1:2], inv_b, m22, OP.mult, OP.subtract)
        nc.vector.reciprocal(inv2, var2)
        nc.scalar.activation(rstd2, inv2, ACTF.Sqrt)
        nc.vector.tensor_tensor(sc2, g2_s, rstd2, OP.mult)
        nc.vector.tensor_tensor(ms2, mean2, sc2, OP.mult)
        nc.vector.tensor_tensor(of2, bb2_s, ms2, OP.subtract)

        # raw logits only; bias b3 + sigmoid are applied on the host
        a2_s = mlp_pool.tile([M2, sl], BF16, tag="v2a")
        y_s = mlp_pool.tile([1, sl], F32, tag="y")
        for a, b in col_tiles():
            nc.scalar.activation(a2_s[:, a:b], v2_s[:, a:b], ACTF.Relu,
                                 bias=of2, scale=sc2)
            pv = ps_a.tile([128, 512], F32, tag="p1")
            nc.tensor.matmul(pv[0:1, :b - a], mw3t_s, a2_s[:, a:b],
                             start=True, stop=True)
            nc.vector.tensor_copy(y_s[:, a:b], pv[0:1, :b - a])
        nc.sync.dma_start(y_out[:], y_s[:])

    nc.compile()
    return nc


_CACHE = {}


def kernel(**inputs) -> np.ndarray:
    x = np.asarray(inputs["x_set"], np.float32)[0]        # [16, T]
    x_scalar = np.asarray(inputs["x_scalar"], np.float32)  # [B, 8]
    lengths = np.asarray(inputs["sample_indices"])[0].astype(np.int64)
    b_total = x_scalar.shape[0]
    offsets = np.concatenate([[0], np.cumsum(lengths)[:-1]])

    groups, ng = _plan_groups(lengths, b_total)
    sl = ng * SLOTS
    per_core = [
        _prep_core(x, x_scalar, lengths, offsets, groups[c], ng)
        for c in range(N_CORES)
    ]

    b1 = np.asarray(inputs["phi_b1"], np.float32)
    b2 = np.asarray(inputs["phi_b2"], np.float32)
    assert np.all(b1 == 0.0) and np.all(b2 == 0.0), \
        "nonzero phi bias path not implemented"

    cb = np.zeros((128, 449), dtype=np.float32)
    w1t = np.asarray(inputs["phi_w1"], np.float32).T      # [16, 64]
    w2t = np.asarray(inputs["phi_w2"], np.float32).T      # [64, 64]
    cb[0:F, 0:F] = w2t
    cb[F:128, F:128] = w2t
    cb[0:C_IN, 128:128 + F] = w1t
    cb[C_IN:2 * C_IN, 128 + F:256] = w1t
    cb[0:F + S_SCALAR, 256:384] = np.asarray(inputs["mlp_w1"], np.float32).T
    cb[0:M1, 384:448] = np.asarray(inputs["mlp_w2"], np.float32).T
    cb[0:M2, 448:449] = np.asarray(inputs["mlp_w3"], np.float32).T

    cf = np.zeros((128, 5), dtype=np.float32)
    cf[:, 0] = np.asarray(inputs["bn1_g"], np.float32)
    cf[:, 1] = np.asarray(inputs["bn1_b"], np.float32)
    cf[0:M2, 2] = np.asarray(inputs["bn2_g"], np.float32)
    cf[0:M2, 3] = np.asarray(inputs["bn2_b"], np.float32)
    cf[0, 4] = float(np.asarray(inputs["mlp_b3"], np.float32).reshape(()))

    consts = {
        "cb": np.ascontiguousarray(cb.astype(ml_dtypes.bfloat16)),
        "cf": np.ascontiguousarray(cf),
    }

    key = (ng, sl, b_total)
    if key not in _CACHE:
        _CACHE[key] = _build_nc(ng, sl, b_total)
    nc = _CACHE[key]

    in_maps = []
    for pc in per_core:
        m = {"xb": pc["xb"], "S": pc["S"], "xsT": pc["xsT"]}
        m.update(consts)
        in_maps.append(m)

    res = bass_utils.run_bass_kernel_spmd(
        nc, in_maps, core_ids=list(range(N_CORES)))

    b3 = float(np.asarray(inputs["mlp_b3"], np.float32).reshape(()))
    y = np.zeros((b_total, 1), dtype=np.float32)
    for c, pc in enumerate(per_core):
        ys = res.results[c]["y"][0]
        se = pc["slot_events"]
        mask = se >= 0
        y[se[mask], 0] = ys[mask]
    y = 1.0 / (1.0 + np.exp(-(y + b3)))
    return y.astype(np.float32)
